# revision 1
# baseline (speedup 1.0000x reference)
"""Causal self-attention (GQA, qk-RMS-norm, RoPE) Trainium2 Bass kernel.

Sharding (8 cores): batch (2) x kv-head-group (4).  Core c handles batch
b = c // 4 and kv head g = c % 4 (with its 4 query heads 4g..4g+3).
Each core computes a (T, D) partial of the output projection (Wproj input
dim is split across the 4 tensor-parallel cores); the host sums the 4
partials per batch element (partials are written fp16; the host
accumulates in f32).

v2 design (everything fp16 device-side; f32 only in PSUM and stats):
  - QKV proj: psum[t_tile, 384] = sum_k xT_tile^T . wqkvT_tile
  - rms stats on raw q/k; the rsqrt runs entirely on DVE (bit-trick seed
    + 2 Newton steps) so the ACT engine only ever needs the Exp table;
    per-head rstd_q * q_gain * SCALE and rstd_k are folded into the fp16
    cast of q/k (gpsimd), so the exp needs no scale operand at all
  - rope applied to raw q/k in fp16 (DVE 2x mode)
  - scores computed transposed S^T[tk, tq] (no softmax max pass needed:
    |s| <= 8 after rms norm), exp straight out of PSUM on ACT,
    multiplicative {0,1} tri-mask on the diagonal 128-block on gpsimd
  - A.V in the y = [tq, d] orientation: one matmul per (head, tq-128,
    tk-tile) with rhs = [V | ones] (65 cols) accumulating y AND the
    softmax denominator in the same psum bank; head-major order because
    PSUM start zeroes the whole 2KB bank lazily.  This replaces the old
    yT-orientation A.V + separate denominator matmuls (~55us less PE)
  - normalization is a per-partition scalar multiply (reciprocal of the
    den column) on DVE, then y head-pairs are PE-transposed to yT
    feeding the output projection
  - kT lives in both 64-partition halves (score matmuls use PE row
    groups for the 2 heads of a pair); replicated by a second PE
    transpose instead of a DMA
  - groups are woven into the chunk stream (chunk c only needs groups
    <= c): qkv psum rides the score-tile slots, transposes ride the
    m-pool slots, so the group-phase DVE/Pool work overlaps chunk
    exp/matmul work
  - DMAs are batched: one xT load per t-tile group, one fp16 output
    store per t-tile.
"""

import os
import sys

import numpy as np

for _p in ("/opt/trn_rl_repo", "/root/.axon_site/_ro/trn_rl_repo"):
    if _p not in sys.path and os.path.isdir(_p):
        sys.path.append(_p)

import concourse.bass as bass
import concourse.bacc as bacc_mod
import concourse.mybir as mybir
import concourse.tile as tile
from concourse.bass import ts
from concourse.bass_utils import run_bass_kernel_spmd

F32 = mybir.dt.float32
F16 = mybir.dt.float16
I32 = mybir.dt.int32

B, T, D = 2, 2048, 1024
H, HKV, HD = 16, 4, 64
NH = H // HKV            # 4 local q heads per core
P = 128
NT = T // P              # 16 t tiles
ND = D // P              # 8 d tiles
GT = 4                   # t-tiles per qkv group
NG = NT // GT            # 4 groups
CW = 512                 # tq chunk width
NCHUNK = T // CW         # 4
QKV_W = NH * HD + 2 * HD  # 384 = q(256) | k(64) | v(64)
NQK = GT * (NH + 1)      # 20 rms/rope lanes per group
EPS = float(np.finfo(np.float32).eps)
SCALE = float(HD) ** -0.5
ROPE_BASE = 10000.0
RSQRT_MAGIC = 0x5F3759DF

# consts layout (f32): [0:20] per-lane rstd fold factor
#   q lanes: q_gain * SCALE * sqrt(HD) = q_gain;  k lanes: sqrt(HD) = 8
# (the DVE rsqrt computes 1/sqrt(ssq + HD*eps) = rstd/sqrt(HD))
NCONST = NQK

# blk layout (fp16): [0:128] identity, [128:640] tri {0,1} mask x4 heads
BLK_W = 640


def _build_bass():
    nc = bacc_mod.Bacc(trn_type="TRN2")

    xT_d = nc.dram_tensor("xT", [D, T], F16, kind="ExternalInput")
    wqkv_d = nc.dram_tensor("wqkvT", [D, QKV_W], F16, kind="ExternalInput")
    wproj_d = nc.dram_tensor("wprojT", [NH * HD, D], F16, kind="ExternalInput")
    # rope tables pre-shuffled host-side to [P, NT, 2, 5, 32] (cos|sin in
    # one tensor, one DMA) so the DMA moves >=512B contiguous runs
    rope_d = nc.dram_tensor(
        "ropeT", [P, 2, NT, NH + 1, HD // 2], F16, kind="ExternalInput"
    )
    # blk carries the f32 qgk consts bitcast into its last 2*NCONST cols
    blk_d = nc.dram_tensor("blk", [P, BLK_W + 2 * NCONST], F16,
                           kind="ExternalInput")
    out_d = nc.dram_tensor("outp", [T, D], F16, kind="ExternalOutput")

    with tile.TileContext(nc) as tc:
        with (
            tc.tile_pool(name="singles", bufs=1) as singles,
            tc.tile_pool(name="xg", bufs=4) as xg_pool,
            tc.tile_pool(name="qk", bufs=2) as qk_pool,
            tc.tile_pool(name="stat", bufs=2) as stat_pool,
            tc.tile_pool(name="u", bufs=16) as u_pool,
            tc.tile_pool(name="r", bufs=2) as r_pool,
            tc.tile_pool(name="ysb", bufs=2) as ysb_pool,
            tc.tile_pool(name="ob", bufs=2) as ob_pool,
            tc.tile_pool(name="s_ps", bufs=2, space="PSUM") as s_pool,
            tc.tile_pool(name="y_ps", bufs=2, space="PSUM") as y_pool,
            tc.tile_pool(name="m_ps", bufs=2, space="PSUM") as m_pool,
        ):
            # ---------------- persistent SBUF ----------------
            wqkv_sb = singles.tile([P, ND, QKV_W], F16)
            wproj_sb = singles.tile([P, 2, D], F16)
            rope_sb = singles.tile([P, 2, NT, NH + 1, HD // 2], F16)
            blk_sb = singles.tile([P, BLK_W + 2 * NCONST], F16)
            magic_sb = singles.tile([P, NQK], I32)
            # pair pr: head 2pr at partitions 0:64, head 2pr+1 at 64:128
            qT_sb = singles.tile([P, 2, T], F16)
            kT_sb = singles.tile([P, T], F16)      # replicated into both halves
            v_sb = singles.tile([P, NT, 66], F16)  # cols 0:64 V, col 64 ones
            yT_sb = singles.tile([P, 2, T], F16)

            # DMA order matters: the transfer device serializes, and the
            # first QKV matmuls need wqkv + xg0 (issued in emit_group(0))
            nc.sync.dma_start(
                out=wqkv_sb, in_=wqkv_d[:].rearrange("(po pi) f -> pi po f", pi=P)
            )
            nc.gpsimd.memset(v_sb, 0.0)
            nc.gpsimd.memset(v_sb[:, :, 64:65], 1.0)
            nc.gpsimd.memset(magic_sb, RSQRT_MAGIC)

            def emit_early_dmas():
                # after xg0: blk (transposes ~12us), rope (rope ~12us)
                nc.sync.dma_start(out=blk_sb, in_=blk_d[:])
                nc.sync.dma_start(out=rope_sb, in_=rope_d[:])

            def emit_late_dmas():
                # wproj is first needed by proj(0), much later
                nc.sync.dma_start(
                    out=wproj_sb,
                    in_=wproj_d[:].rearrange("(po pi) f -> pi po f", pi=P),
                )

            qgk = blk_sb[:, BLK_W : BLK_W + 2 * NCONST].bitcast(F32)
            ident = blk_sb[:, 0:128]
            tri4 = blk_sb[:, 128:640].rearrange("p (j f) -> p j f", j=4)

            # PE p-state warmup: the tensor engine ramps to full clock only
            # after ~3us of continuous execution, and the cost of a matmul is
            # locked at dispatch.  Junk matmuls (no DMA dependency) keep PE
            # busy from t~0.7us until the first xg tile lands, so all real
            # matmuls price at the full clock.
            warm_sb = singles.tile([P, CW], F16)
            nc.gpsimd.memset(warm_sb, 0.0)

            def emit_warmup(n):
                wps = s_pool.tile([P, 2, CW], F32, tag="s", name="warm")
                for i in range(n):
                    nc.tensor.matmul(
                        wps[:, i % 2, :],
                        lhsT=warm_sb[:, 0:128],
                        rhs=warm_sb,
                        start=True,
                        stop=True,
                        skip_group_check=True,
                    )

            xg_tiles = {}

            def emit_xg(g):
                xg_sb = xg_pool.tile([P, ND, GT * P], F16, tag="xg", name=f"xg{g}")
                nc.sync.dma_start(
                    out=xg_sb,
                    in_=xT_d[:, ts(g, GT * P)].rearrange(
                        "(po pi) t -> pi po t", pi=P
                    ),
                )
                xg_tiles[g] = xg_sb

            def emit_group(g):
                """QKV projection + rms stats + rope + fp16 cast (with the
                rstd/q_gain/SCALE folds) + transposes for group g.  QKV psum
                rides the score-tile slots; transposes ride the m slots."""
                xg_sb = xg_tiles[g]
                qk_raw = qk_pool.tile([P, NQK, HD], F16, tag="qkraw")
                sq = qk_pool.tile([P, NQK, HD], F16, tag="sq")
                for half in range(2):
                    qkv_ps = s_pool.tile([P, 2, 512], F32, tag="s",
                                         name=f"qkv{g}_{half}")
                    for jj in range(2):
                        j = 2 * half + jj
                        for ik in range(ND):
                            nc.tensor.matmul(
                                qkv_ps[:, jj, 0:QKV_W],
                                lhsT=xg_sb[:, ik, ts(j, P)],
                                rhs=wqkv_sb[:, ik, :],
                                start=(ik == 0),
                                stop=(ik == ND - 1),
                            )
                    h0 = 2 * half * (NH + 1)
                    nc.vector.tensor_copy(
                        qk_raw[:, h0 : h0 + 2 * (NH + 1), :].rearrange(
                            "p n x -> p (n x)"
                        ),
                        qkv_ps[:, :, 0 : (NH + 1) * HD],
                    )
                    # squares on ACT (Square shares the Exp table) straight
                    # from PSUM: takes stats off the rope critical path
                    nc.scalar.square(
                        sq[:, h0 : h0 + 2 * (NH + 1), :].rearrange(
                            "p n x -> p (n x)"
                        ),
                        qkv_ps[:, :, 0 : (NH + 1) * HD],
                    )
                    nc.vector.tensor_copy(
                        v_sb[:, ts(2 * g + half, 2), 0:64],
                        qkv_ps[:, :, 320:384],
                    )

                ssq = stat_pool.tile([P, NQK], F32, tag="ssq")
                nc.vector.reduce_sum(ssq, sq, axis=mybir.AxisListType.X)
                nc.vector.tensor_scalar_add(
                    out=ssq, in0=ssq, scalar1=float(HD * EPS)
                )
                # rstd/sqrt(HD) = rsqrt(ssq): bit-trick seed + 2 Newton steps
                # (keeps ACT pinned to the Exp table: no table reloads)
                rstd = stat_pool.tile([P, NQK], F32, tag="rstd")
                nt_ = stat_pool.tile([P, NQK], F32, tag="nt")
                rstd_i = rstd[:, :].bitcast(I32)
                nc.vector.tensor_scalar(
                    out=rstd_i,
                    in0=ssq[:, :].bitcast(I32),
                    scalar1=1,
                    scalar2=None,
                    op0=mybir.AluOpType.logical_shift_right,
                )
                nc.vector.tensor_sub(rstd_i, magic_sb, rstd_i)
                for _ in range(2):
                    nc.vector.tensor_mul(nt_, rstd, rstd)
                    nc.vector.tensor_mul(nt_, nt_, ssq)
                    nc.vector.tensor_scalar(
                        out=nt_,
                        in0=nt_,
                        scalar1=-0.5,
                        scalar2=1.5,
                        op0=mybir.AluOpType.mult,
                        op1=mybir.AluOpType.add,
                    )
                    nc.vector.tensor_mul(rstd, rstd, nt_)
                # fold q_gain (q lanes) / sqrt(HD) restore (all lanes)
                nc.vector.tensor_mul(rstd, rstd, qgk)

                # rope in place on raw q|k (rotation commutes with rms scale)
                q1 = qk_raw[:, :, 0 : HD // 2]
                q2 = qk_raw[:, :, HD // 2 : HD]
                cg = rope_sb[:, 0:1, ts(g, GT), :, :].rearrange(
                    "p o g h x -> p (o g h) x"
                )
                sg = rope_sb[:, 1:2, ts(g, GT), :, :].rearrange(
                    "p o g h x -> p (o g h) x"
                )
                t_a = qk_pool.tile([P, NQK, HD // 2], F16, tag="ta")
                t_b = qk_pool.tile([P, NQK, HD // 2], F16, tag="tb")
                t_c = qk_pool.tile([P, NQK, HD // 2], F16, tag="tc")
                t_d = qk_pool.tile([P, NQK, HD // 2], F16, tag="td")
                nc.vector.tensor_mul(t_a, q1, cg)
                nc.vector.tensor_mul(t_b, q2, sg)
                nc.vector.tensor_mul(t_c, q1, sg)
                nc.vector.tensor_mul(t_d, q2, cg)
                nc.vector.tensor_add(q1, t_a, t_b)
                nc.vector.tensor_sub(q2, t_d, t_c)

                # scale q heads by rstd*gain*SCALE and k by rstd_k (fp16);
                # first half on DVE (shortest latency to the first
                # transposes), second half offloaded to gpsimd
                qk_c = qk_pool.tile([P, NQK, HD], F16, tag="qkc")
                for i in range(NQK):
                    eng = nc.vector if i < NQK // 2 else nc.gpsimd
                    eng.tensor_scalar_mul(
                        out=qk_c[:, i, :],
                        in0=qk_raw[:, i, :],
                        scalar1=rstd[:, i : i + 1],
                    )

                # fp16 transposes: q head-pairs; k twice (both 64-part halves)
                for j in range(GT):
                    it = g * GT + j
                    i0 = j * (NH + 1)
                    for pr in range(2):
                        trq = m_pool.tile([P, 2, CW], F16, tag="m",
                                          name=f"trq{it}_{pr}")
                        nc.tensor.transpose(
                            trq[:, 0, 0:128],
                            qk_c[:, i0 + 2 * pr : i0 + 2 * pr + 2, :],
                            ident,
                        )
                        nc.vector.tensor_copy(
                            qT_sb[:, pr, ts(it, P)], trq[:, 0, 0:128]
                        )
                    trk = m_pool.tile([P, 2, CW], F16, tag="m", name=f"trk{it}")
                    nc.tensor.transpose(
                        trk[0:64, 0, 0:128], qk_c[:, i0 + NH, :], ident
                    )
                    nc.tensor.transpose(
                        trk[64:128, 0, 0:128], qk_c[:, i0 + NH, :], ident
                    )
                    nc.vector.tensor_copy(kT_sb[:, ts(it, P)], trk[:, 0, 0:128])

            def emit_scores(c, tk, u_tiles):
                dj = tk - 4 * c  # >= 0 on the diagonal tiles
                lo = P * dj if dj >= 0 else 0
                u = u_pool.tile([P, NH, CW], F16, tag="u",
                                name=f"u_c{c}_{tk}")
                for pr in range(2):
                    s_ps = s_pool.tile([P, 2, CW], F32, tag="s")
                    for hh in range(2):
                        nc.tensor.matmul(
                            s_ps[:, hh, lo:],
                            lhsT=kT_sb[64 * hh : 64 * (hh + 1), ts(tk, P)],
                            rhs=qT_sb[
                                64 * hh : 64 * (hh + 1),
                                pr,
                                c * CW + lo : (c + 1) * CW,
                            ],
                            start=True,
                            stop=True,
                        )
                    nc.scalar.activation(
                        out=u[:, 2 * pr : 2 * pr + 2, lo:],
                        in_=s_ps[:, :, lo:],
                        func=mybir.ActivationFunctionType.Exp,
                    )
                if dj >= 0:
                    # multiplicative causal mask on the diagonal 128-block
                    nc.gpsimd.tensor_mul(
                        u[:, :, lo : lo + P], u[:, :, lo : lo + P], tri4
                    )
                u_tiles.append(u)

            def emit_av(c, s, u_tiles):
                """A.V + normalize + yT transpose for sub-chunk s (t-tile
                it = 4c+s); needs u tiles 0..4c+s."""
                it = c * GT + s
                nk = 4 * c + s + 1
                # y tile padded to a full 2KB psum bank: PSUM start zeroing
                # is lazy per whole bank
                y_ps = y_pool.tile([P, NH, 128], F32, tag="y",
                                   name=f"y_c{c}_{s}")
                # head-major: each head's full accumulation chain before the
                # next head's start
                for h in range(NH):
                    for tk in range(nk):
                        nc.tensor.matmul(
                            y_ps[:, h, 0:65],
                            lhsT=u_tiles[tk][:, h, ts(s, P)],
                            rhs=v_sb[:, tk, 0:65],
                            start=(tk == 0),
                            stop=(tk == nk - 1),
                            skip_group_check=True,
                        )
                dr = r_pool.tile([P, NH], F32, tag="dr")
                nc.vector.reciprocal(dr, y_ps[:, :, 64:65])
                y_sb = ysb_pool.tile([P, NH, HD], F16, tag="ysb")
                for h in range(NH):
                    nc.vector.tensor_scalar_mul(
                        out=y_sb[:, h, :],
                        in0=y_ps[:, h, 0:64],
                        scalar1=dr[:, h : h + 1],
                    )
                for pr in range(2):
                    try_ = m_pool.tile([P, 2, CW], F16, tag="m",
                                       name=f"try_{it}_{pr}")
                    nc.tensor.transpose(
                        try_[:, 0, 0:128],
                        y_sb[:, 2 * pr : 2 * pr + 2, :],
                        ident,
                    )
                    nc.vector.tensor_copy(
                        yT_sb[:, pr, ts(it, P)], try_[:, 0, 0:128]
                    )

            def emit_proj_tile(it):
                """Output projection for t-tile it -> fp16 partial."""
                ob = ob_pool.tile([P, D], F16, tag="ob")
                for nh_ in range(2):
                    pj = m_pool.tile([P, CW], F32, tag="m",
                                     name=f"pj{it}_{nh_}")
                    for kt in range(2):
                        nc.tensor.matmul(
                            pj,
                            lhsT=yT_sb[:, kt, ts(it, P)],
                            rhs=wproj_sb[:, kt, ts(nh_, CW)],
                            start=(kt == 0),
                            stop=(kt == 1),
                        )
                    nc.vector.tensor_copy(ob[:, ts(nh_, CW)], pj)
                nc.sync.dma_start(out=out_d[ts(it, P), :], in_=ob)

            def emit_chunk(c):
                """Attention for tq chunk c (needs groups 0..c done).  The
                AV/normalize/proj work for sub-chunk s is woven in right
                after exp(tk=4c+s) so PE never waits for the whole exp
                stream, and the previous t-tile's projection rides along."""
                ntk = (c + 1) * (CW // P)
                u_tiles = []
                for tk in range(4 * c):
                    emit_scores(c, tk, u_tiles)
                for s in range(GT):
                    emit_scores(c, 4 * c + s, u_tiles)
                    emit_av(c, s, u_tiles)
                    it = c * GT + s
                    if it > 0:
                        emit_proj_tile(it - 1)

            emit_warmup(16)
            emit_xg(0)
            emit_early_dmas()
            emit_xg(1)
            emit_group(0)
            emit_xg(2)
            emit_xg(3)
            emit_group(1)
            emit_late_dmas()
            emit_chunk(0)
            emit_group(2)
            emit_chunk(1)
            emit_group(3)
            emit_chunk(2)
            emit_chunk(3)
            emit_proj_tile(NT - 1)

    nc.finalize()
    return nc


_NC_CACHE = {}


def _get_nc():
    if "nc" not in _NC_CACHE:
        _NC_CACHE["nc"] = _build_bass()
    return _NC_CACHE["nc"]


def _make_blk(q_gain_local):
    blk = np.zeros((P, BLK_W + 2 * NCONST), dtype=np.float16)
    blk[:, 0:128] = np.eye(P, dtype=np.float32)
    tri = (np.arange(P)[None, :] >= np.arange(P)[:, None]).astype(np.float32)
    for j in range(4):
        blk[:, 128 + 128 * j : 256 + 128 * j] = tri
    # per-lane rstd fold factor (f32, bitcast into the f16 tail):
    # q lanes: q_gain * SCALE * sqrt(HD) = q_gain;  k lanes: sqrt(HD)
    lane = np.empty((NQK,), np.float32)
    for j in range(GT):
        lane[j * (NH + 1) : j * (NH + 1) + NH] = np.asarray(
            q_gain_local, np.float32
        )
        lane[j * (NH + 1) + NH] = np.sqrt(HD)
    qgk = np.broadcast_to(lane[None, :], (P, NQK)).astype(np.float32)
    blk[:, BLK_W : BLK_W + 2 * NCONST] = np.ascontiguousarray(qgk).view(
        np.float16
    )
    return blk


def _rope_tables():
    inv = 1.0 / (ROPE_BASE ** (np.arange(0, HD, 2, dtype=np.float32) / HD))
    f = np.arange(T, dtype=np.float32)[:, None] * inv[None, :]
    # replicate across the 4 q heads + 1 k head (no zero-stride broadcast
    # APs in TensorTensor), pre-shuffled to the [P, NT, 2, 5, 32] SBUF
    # layout (cos|sin in one tensor) so the DMA is one contiguous run per
    # partition
    def shuf(a):
        a5 = np.broadcast_to(a[:, None, :], (T, NH + 1, HD // 2))
        # [T=(nt p), h, f] -> [p, nt, h, f]
        return a5.reshape(NT, P, NH + 1, HD // 2).transpose(1, 0, 2, 3)

    rope = np.stack([shuf(np.cos(f)), shuf(np.sin(f))], axis=1)
    return np.ascontiguousarray(rope).astype(np.float16)


def _make_in_maps(x, Wq, Wk, Wv, Wproj, q_gain):
    x = np.ascontiguousarray(np.asarray(x, np.float32))
    Wq = np.asarray(Wq, np.float32)
    Wk = np.asarray(Wk, np.float32)
    Wv = np.asarray(Wv, np.float32)
    Wproj = np.asarray(Wproj, np.float32)
    q_gain = np.asarray(q_gain, np.float32)
    rope = _rope_tables()
    xTs = [np.ascontiguousarray(x[b].T.astype(np.float16)) for b in range(B)]
    kvw = NH * HD  # 256 per-core q slice width
    in_maps = []
    for core in range(8):
        b, g = divmod(core, HKV)
        wq = Wq[g * kvw : (g + 1) * kvw]
        wk = Wk[g * HD : (g + 1) * HD]
        wv = Wv[g * HD : (g + 1) * HD]
        wqkvT = np.ascontiguousarray(
            np.concatenate([wq, wk, wv], 0).T.astype(np.float16)
        )
        wprojT = np.ascontiguousarray(
            Wproj[:, g * kvw : (g + 1) * kvw].T.astype(np.float16)
        )
        in_maps.append(
            {
                "xT": xTs[b],
                "wqkvT": wqkvT,
                "wprojT": wprojT,
                "ropeT": rope,
                "blk": _make_blk(q_gain[g * NH : (g + 1) * NH]),
            }
        )
    return in_maps


def run_sharded(inputs, trace=False, **kwargs):
    """Run the SPMD kernel; returns (full_output, BassKernelResults)."""
    in_maps = _make_in_maps(**inputs)
    res = run_bass_kernel_spmd(
        _get_nc(), in_maps, core_ids=list(range(8)), trace=trace, **kwargs
    )
    out = np.zeros((B, T, D), np.float32)
    for core in range(8):
        out[core // HKV] += res.results[core]["outp"].astype(np.float32)
    return out, res


def kernel(x, Wq, Wk, Wv, Wproj, q_gain):
    out, _ = run_sharded(
        dict(x=x, Wq=Wq, Wk=Wk, Wv=Wv, Wproj=Wproj, q_gain=q_gain)
    )
    return out



# revision 30
# speedup vs baseline: 1.1717x; 1.1717x over previous
"""Causal self-attention (GQA, qk-RMS-norm, RoPE) Trainium2 Bass kernel.

Sharding (8 cores): batch (2) x kv-head-group (4).  Core c handles batch
b = c // 4 and kv head g = c % 4 (with its 4 query heads 4g..4g+3).
Each core computes a (T, D) partial of the output projection (Wproj input
dim is split across the 4 tensor-parallel cores); the host sums the 4
partials per batch element (partials are written fp16; the host
accumulates in f32).

v4 design (everything fp16 device-side; f32 only in PSUM and stats):
  - QKV proj: psum[t_tile, 384] = sum_k xT_tile^T . wqkvT_tile
  - rms stats on raw q/k; rsqrt via DVE bit-trick + 2 Newton steps;
    per-head rstd * q_gain * SCALE folded into the fp16 cast of q/k
  - rope applied to raw q/k in fp16 (DVE 2x mode)
  - scores computed transposed S^T[tk, tq]; exp straight out of PSUM on
    ACT; multiplicative {0,1} tri-mask on the diagonal 128-block
  - k is cast into TWO adjacent lanes so a single [128,128] PE transpose
    lands kT duplicated across both partition halves
  - per-group transposes are packed into one psum bank and drained by a
    single batched DVE copy
  - A.V with rhs = [V | ones] accumulating y and the softmax denominator
  - output projection per 128-row tile; psum evacuated by DVE/ACT copies
  - schedule: group fronts/transposes woven between chunk score/AV
    streams so ACT exp is continuously fed
"""

import os
import sys

import numpy as np

for _p in ("/opt/trn_rl_repo", "/root/.axon_site/_ro/trn_rl_repo"):
    if _p not in sys.path and os.path.isdir(_p):
        sys.path.append(_p)

import concourse.bass as bass
import concourse.bacc as bacc_mod
import concourse.mybir as mybir
import concourse.tile as tile
from concourse.bass import ts
from concourse.bass_utils import run_bass_kernel_spmd

F32 = mybir.dt.float32
F16 = mybir.dt.float16
I16 = mybir.dt.int16
I32 = mybir.dt.int32

B, T, D = 2, 2048, 1024
H, HKV, HD = 16, 4, 64
NH = H // HKV            # 4 local q heads per core
P = 128
NT = T // P              # 16 t tiles
ND = D // P              # 8 d tiles
GT = 4                   # t-tiles per qkv group
NG = NT // GT            # 4 groups
CW = 512                 # tq chunk width
NCHUNK = T // CW         # 4
QKV_W = NH * HD + 2 * HD  # 384 = q(256) | k(64) | v(64)
NQK = GT * (NH + 1)      # 20 rms/rope lanes per group
EPS = float(np.finfo(np.float32).eps)
SCALE = float(HD) ** -0.5
ROPE_BASE = 10000.0
RSQRT_MAGIC = 0x5F3759DF

NCONST = NQK
BLK_W = 640

# ---- engine-assignment knobs (tuned against the TimelineSim cost model) ----
CFG = {
    "warm": 10,
    "rope_pool_groups": (),      # groups whose rope runs on gpsimd
    "mask_pool_chunks": (),      # chunks whose tri-mask runs on gpsimd
    "ob_act_tiles": (),          # t-tiles whose first ob half copies on ACT
    "ynorm_act_tiles": (),       # t-tiles whose y-normalize runs on ACT
    # chunks (by t0) whose pr=1 exp runs as the DVE fp16 int-exp
    # (2^x bit trick; ~2% element rms err on those weights)
    "intexp_chunks": (12,),
}

# int-exp constants: i16 = s * 1024*log2e + (15360 - 44 + 0.5) -> f16 bits
INTEXP_C = 1024.0 / float(np.log(2.0))
INTEXP_B = 15316.5


def _build_bass():
    nc = bacc_mod.Bacc(trn_type="TRN2")

    xT_d = nc.dram_tensor("xT", [D, T], F16, kind="ExternalInput")
    wqkv_d = nc.dram_tensor("wqkvT", [D, QKV_W], F16, kind="ExternalInput")
    wproj_d = nc.dram_tensor("wprojT", [NH * HD, D], F16, kind="ExternalInput")
    rope_d = nc.dram_tensor(
        "ropeT", [P, 2, NT, NH + 1, HD // 2], F16, kind="ExternalInput"
    )
    blk_d = nc.dram_tensor("blk", [P, BLK_W + 2 * NCONST], F16,
                           kind="ExternalInput")
    out_d = nc.dram_tensor("outp", [T, D], F16, kind="ExternalOutput")

    with tile.TileContext(nc) as tc:
        with (
            tc.tile_pool(name="singles", bufs=1) as singles,
            tc.tile_pool(name="xg", bufs=4) as xg_pool,
            tc.tile_pool(name="qk", bufs=2) as qk_pool,
            tc.tile_pool(name="stat", bufs=2) as stat_pool,
            tc.tile_pool(name="u", bufs=24) as u_pool,
            tc.tile_pool(name="r", bufs=2) as r_pool,
            tc.tile_pool(name="ysb", bufs=2) as ysb_pool,
            tc.tile_pool(name="ob", bufs=2) as ob_pool,
            tc.tile_pool(name="s_ps", bufs=2, space="PSUM") as s_pool,
            tc.tile_pool(name="y_ps", bufs=2, space="PSUM") as y_pool,
            tc.tile_pool(name="m_ps", bufs=2, space="PSUM") as m_pool,
        ):
            # ---------------- persistent SBUF ----------------
            wqkv_sb = singles.tile([P, ND, QKV_W], F16)
            wproj_sb = singles.tile([P, 2, D], F16)
            rope_sb = singles.tile([P, 2, NT, NH + 1, HD // 2], F16)
            blk_sb = singles.tile([P, BLK_W + 2 * NCONST], F16)
            magic_sb = singles.tile([P, NQK], I32)
            # pair pr: head 2pr at partitions 0:64, head 2pr+1 at 64:128
            qT_sb = singles.tile([P, 2, T], F16)
            kT_sb = singles.tile([P, T], F16)      # dup'd into both halves
            v_sb = singles.tile([P, NT, 66], F16)  # cols 0:64 V, col 64 ones
            yT_sb = singles.tile([P, 2, T], F16)

            nc.sync.dma_start(
                out=wqkv_sb, in_=wqkv_d[:].rearrange("(po pi) f -> pi po f", pi=P)
            )
            nc.gpsimd.memset(v_sb, 0.0)
            nc.gpsimd.memset(v_sb[:, :, 64:65], 1.0)
            nc.gpsimd.memset(magic_sb, RSQRT_MAGIC)

            def emit_early_dmas():
                nc.sync.dma_start(out=blk_sb, in_=blk_d[:])
                nc.sync.dma_start(out=rope_sb, in_=rope_d[:])

            def emit_late_dmas():
                nc.sync.dma_start(
                    out=wproj_sb,
                    in_=wproj_d[:].rearrange("(po pi) f -> pi po f", pi=P),
                )

            qgk = blk_sb[:, BLK_W : BLK_W + 2 * NCONST].bitcast(F32)
            ident = blk_sb[:, 0:128]
            tri4 = blk_sb[:, 128:640].rearrange("p (j f) -> p j f", j=4)

            # PE p-state warmup: junk matmuls keep PE busy until the first
            # xg tile lands so all real matmuls price at the full clock.
            warm_sb = singles.tile([P, CW], F16)
            nc.gpsimd.memset(warm_sb, 0.0)
            # primer: forces the ACT exp-table load at t~0 instead of right
            # before the first real exp
            prime_sb = singles.tile([P, 1], F16)
            nc.scalar.square(prime_sb, warm_sb[:, 0:1])

            def emit_warmup(n):
                wps = s_pool.tile([P, 2, CW], F32, tag="s", name="warm")
                for i in range(n):
                    nc.tensor.matmul(
                        wps[:, i % 2, :],
                        lhsT=warm_sb[:, 0:128],
                        rhs=warm_sb,
                        start=True,
                        stop=True,
                        skip_group_check=True,
                    )

            xg_tiles = {}

            def emit_xg(g):
                xg_sb = xg_pool.tile([P, ND, GT * P], F16, tag="xg", name=f"xg{g}")
                nc.sync.dma_start(
                    out=xg_sb,
                    in_=xT_d[:, ts(g, GT * P)].rearrange(
                        "(po pi) t -> pi po t", pi=P
                    ),
                )
                xg_tiles[g] = xg_sb

            qkc_tiles = {}

            def emit_group_front(g, halves=(0, 1)):
                """QKV projection + rms stats + rope + fp16 cast (with the
                rstd/q_gain/SCALE folds) for group g (optionally one half =
                2 t-tiles at a time for head-latency pipelining)."""
                xg_sb = xg_tiles[g]
                if 0 in halves:
                    qkc_tiles[g] = qk_pool.tile(
                        [P, GT, NH + 2, HD], F16, tag="qkc",
                        name=f"qkc{g}"
                    )
                    raw_tiles[g] = (
                        qk_pool.tile([P, NQK, HD], F16, tag="qkraw",
                                     name=f"qkraw{g}"),
                        qk_pool.tile([P, NQK, HD], F16, tag="sq",
                                     name=f"sq{g}"),
                        stat_pool.tile([P, NQK], F32, tag="ssq",
                                       name=f"ssq{g}"),
                        stat_pool.tile([P, NQK], F32, tag="rstd",
                                       name=f"rstd{g}"),
                        stat_pool.tile([P, NQK], F32, tag="nt",
                                       name=f"nt{g}"),
                    )
                qk_c = qkc_tiles[g]
                qk_raw, sq, ssq_f, rstd_f, nt_f = raw_tiles[g]
                for half in halves:
                    qkv_ps = s_pool.tile([P, 2, 512], F32, tag="s",
                                         name=f"qkv{g}_{half}")
                    for jj in range(2):
                        j = 2 * half + jj
                        for ik in range(ND):
                            nc.tensor.matmul(
                                qkv_ps[:, jj, 0:QKV_W],
                                lhsT=xg_sb[:, ik, ts(j, P)],
                                rhs=wqkv_sb[:, ik, :],
                                start=(ik == 0),
                                stop=(ik == ND - 1),
                            )
                    h0 = 2 * half * (NH + 1)
                    hn = 2 * (NH + 1)
                    ceng = nc.scalar if g < 2 else nc.vector
                    if g < 2:
                        nc.scalar.copy(
                            qk_raw[:, h0 : h0 + hn, :].rearrange(
                                "p n x -> p (n x)"
                            ),
                            qkv_ps[:, :, 0 : (NH + 1) * HD],
                        )
                    else:
                        nc.vector.tensor_copy(
                            qk_raw[:, h0 : h0 + hn, :].rearrange(
                                "p n x -> p (n x)"
                            ),
                            qkv_ps[:, :, 0 : (NH + 1) * HD],
                        )
                    nc.vector.tensor_copy(
                        v_sb[:, ts(2 * g + half, 2), 0:64],
                        qkv_ps[:, :, 320:384],
                    )

                    # squares on DVE from the fp16 copy (frees the qkv psum
                    # slot sooner and keeps the ACT queue exp-only)
                    nc.vector.tensor_mul(
                        sq[:, h0 : h0 + hn, :], qk_raw[:, h0 : h0 + hn, :],
                        qk_raw[:, h0 : h0 + hn, :],
                    )
                    ssq = ssq_f[:, h0 : h0 + hn]
                    nc.vector.reduce_sum(
                        ssq, sq[:, h0 : h0 + hn, :], axis=mybir.AxisListType.X
                    )
                    nc.vector.tensor_scalar_add(
                        out=ssq, in0=ssq, scalar1=float(HD * EPS)
                    )
                    # rstd/sqrt(HD) = rsqrt(ssq): bit-trick + 2 Newton steps
                    rstd = rstd_f[:, h0 : h0 + hn]
                    nt_ = nt_f[:, h0 : h0 + hn]
                    rstd_i = rstd.bitcast(I32)
                    nc.vector.tensor_scalar(
                        out=rstd_i,
                        in0=ssq.bitcast(I32),
                        scalar1=1,
                        scalar2=None,
                        op0=mybir.AluOpType.logical_shift_right,
                    )
                    nc.vector.tensor_sub(
                        rstd_i, magic_sb[:, h0 : h0 + hn], rstd_i
                    )
                    for _ in range(2):
                        nc.vector.tensor_mul(nt_, rstd, rstd)
                        nc.vector.tensor_mul(nt_, nt_, ssq)
                        nc.vector.tensor_scalar(
                            out=nt_,
                            in0=nt_,
                            scalar1=-0.5,
                            scalar2=1.5,
                            op0=mybir.AluOpType.mult,
                            op1=mybir.AluOpType.add,
                        )
                        nc.vector.tensor_mul(rstd, rstd, nt_)
                    # fold q_gain (q lanes) / sqrt(HD) restore (all lanes)
                    nc.vector.tensor_mul(
                        rstd, rstd, qgk[:, h0 : h0 + hn]
                    )

                    # rope in place on raw q|k
                    rr = qk_raw[:, h0 : h0 + hn, :]
                    q1 = rr[:, :, 0 : HD // 2]
                    q2 = rr[:, :, HD // 2 : HD]
                    tsl = ts(2 * g + half, 2)
                    cg = rope_sb[:, 0:1, tsl, :, :].rearrange(
                        "p o g h x -> p (o g h) x"
                    )
                    sg = rope_sb[:, 1:2, tsl, :, :].rearrange(
                        "p o g h x -> p (o g h) x"
                    )
                    t_a = qk_pool.tile([P, hn, HD // 2], F16, tag="ta",
                                       name=f"ta{g}_{half}")
                    t_b = qk_pool.tile([P, hn, HD // 2], F16, tag="tb",
                                       name=f"tb{g}_{half}")
                    t_c = qk_pool.tile([P, hn, HD // 2], F16, tag="tc",
                                       name=f"tc{g}_{half}")
                    t_d = qk_pool.tile([P, hn, HD // 2], F16, tag="td",
                                       name=f"td{g}_{half}")
                    reng = (nc.gpsimd if g in CFG["rope_pool_groups"]
                            else nc.vector)
                    reng.tensor_mul(t_a, q1, cg)
                    reng.tensor_mul(t_b, q2, sg)
                    reng.tensor_mul(t_c, q1, sg)
                    reng.tensor_mul(t_d, q2, cg)
                    reng.tensor_add(q1, t_a, t_b)
                    reng.tensor_sub(q2, t_d, t_c)

                    # scale into qk_c; k goes to TWO adjacent lanes so one
                    # [128,128] PE transpose lands kT dup'd in both halves
                    ndve = 0
                    for jj in range(2):
                        j = 2 * half + jj
                        i0 = j * (NH + 1)
                        for h in range(NH + 2):
                            src_lane = i0 + min(h, NH)
                            eng = nc.vector if (ndve % 2 == 0) else nc.gpsimd
                            ndve += 1
                            eng.tensor_scalar_mul(
                                out=qk_c[:, j, h, :],
                                in0=qk_raw[:, src_lane, :],
                                scalar1=rstd_f[:, src_lane : src_lane + 1],
                            )

            def emit_group_tr(g, halves=(0, 1)):
                """fp16 transposes: q-pair + dup'd-k transposes packed into
                psum tiles, then ONE batched DVE copy each."""
                qk_c = qkc_tiles[g]
                js = [2 * h + jj for h in halves for jj in range(2)]
                nj = len(js)
                trq = m_pool.tile([P, 2, nj * P], F16, tag="m",
                                  name=f"trq{g}_{halves[0]}")
                trk = m_pool.tile([P, nj * P], F16, tag="m",
                                  name=f"trk{g}_{halves[0]}")
                for i, j in enumerate(js):
                    for pr in range(2):
                        nc.tensor.transpose(
                            trq[:, pr, ts(i, P)],
                            qk_c[:, j, 2 * pr : 2 * pr + 2, :],
                            ident,
                        )
                    nc.tensor.transpose(
                        trk[:, ts(i, P)], qk_c[:, j, NH : NH + 2, :], ident
                    )
                t0 = g * GT + js[0]
                nc.vector.tensor_copy(
                    qT_sb[:, :, t0 * P : (t0 + nj) * P], trq
                )
                nc.vector.tensor_copy(
                    kT_sb[:, t0 * P : (t0 + nj) * P], trk
                )

            def emit_scores(t0, ntl, tk, u_tiles):
                """Score tile (tk) for the tq range [t0, t0+ntl) t-tiles."""
                dj = tk - t0  # >= 0 on the diagonal tiles
                lo = P * dj if dj >= 0 else 0
                w = ntl * P
                u = u_pool.tile([P, NH, CW], F16, tag="u",
                                name=f"u_t{t0}_{tk}")
                for pr in range(2):
                    s_ps = s_pool.tile([P, 2, CW], F32, tag="s")
                    for hh in range(2):
                        nc.tensor.matmul(
                            s_ps[:, hh, lo:w],
                            lhsT=kT_sb[64 * hh : 64 * (hh + 1), ts(tk, P)],
                            rhs=qT_sb[
                                64 * hh : 64 * (hh + 1),
                                pr,
                                t0 * P + lo : (t0 + ntl) * P,
                            ],
                            start=True,
                            stop=True,
                        )
                    if pr == 1 and t0 in CFG["intexp_chunks"]:
                        # exp(s) ~= bitcast_f16(i16(s*1024*log2e + B)):
                        # one DVE op; splits the exp cadence across ACT+DVE
                        nc.vector.tensor_scalar(
                            out=u[:, 2 * pr : 2 * pr + 2, lo:w].bitcast(I16),
                            in0=s_ps[:, :, lo:w],
                            scalar1=INTEXP_C,
                            scalar2=INTEXP_B,
                            op0=mybir.AluOpType.mult,
                            op1=mybir.AluOpType.add,
                        )
                    else:
                        nc.scalar.activation(
                            out=u[:, 2 * pr : 2 * pr + 2, lo:w],
                            in_=s_ps[:, :, lo:w],
                            func=mybir.ActivationFunctionType.Exp,
                        )
                if dj >= 0:
                    # multiplicative causal mask on the diagonal 128-block
                    meng = (nc.gpsimd if t0 in CFG["mask_pool_chunks"]
                            else nc.vector)
                    meng.tensor_mul(
                        u[:, :, lo : lo + P], u[:, :, lo : lo + P], tri4
                    )
                u_tiles.append(u)

            def emit_av(t0, s, u_tiles):
                """A.V + normalize + yT transpose for t-tile it = t0+s."""
                it = t0 + s
                nk = it + 1
                y_ps = y_pool.tile([P, NH, 128], F32, tag="y",
                                   name=f"y_t{it}")
                for h in range(NH):
                    for tk in range(nk):
                        nc.tensor.matmul(
                            y_ps[:, h, 0:65],
                            lhsT=u_tiles[tk][:, h, ts(s, P)],
                            rhs=v_sb[:, tk, 0:65],
                            start=(tk == 0),
                            stop=(tk == nk - 1),
                            skip_group_check=True,
                        )
                dr = r_pool.tile([P, NH], F32, tag="dr")
                nc.vector.reciprocal(dr, y_ps[:, :, 64:65])
                y_sb = ysb_pool.tile([P, NH, HD], F16, tag="ysb")
                for h in range(NH):
                    if it in CFG["ynorm_act_tiles"]:
                        nc.scalar.activation(
                            out=y_sb[:, h, :],
                            in_=y_ps[:, h, 0:64],
                            func=mybir.ActivationFunctionType.Copy,
                            scale=dr[:, h : h + 1],
                        )
                    else:
                        nc.vector.tensor_scalar_mul(
                            out=y_sb[:, h, :],
                            in0=y_ps[:, h, 0:64],
                            scalar1=dr[:, h : h + 1],
                        )
                try_ = m_pool.tile([P, 2, 128], F16, tag="m",
                                   name=f"try_{it}")
                for pr in range(2):
                    nc.tensor.transpose(
                        try_[:, pr, :],
                        y_sb[:, 2 * pr : 2 * pr + 2, :],
                        ident,
                    )
                nc.vector.tensor_copy(yT_sb[:, :, ts(it, P)], try_)

            def emit_proj_tile(it):
                """Output projection for t-tile it -> fp16 partial."""
                ob = ob_pool.tile([P, D], F16, tag="ob")
                for nh_ in range(2):
                    pj = m_pool.tile([P, 512], F32, tag="m",
                                     name=f"pj{it}_{nh_}")
                    for kt in range(2):
                        nc.tensor.matmul(
                            pj,
                            lhsT=yT_sb[:, kt, ts(it, P)],
                            rhs=wproj_sb[:, kt, ts(nh_, 512)],
                            start=(kt == 0),
                            stop=(kt == 1),
                        )
                    if it in CFG["ob_act_tiles"] and nh_ == 0:
                        nc.scalar.copy(ob[:, ts(nh_, 512)], pj)
                    else:
                        nc.vector.tensor_copy(ob[:, ts(nh_, 512)], pj)
                nc.sync.dma_start(out=out_d[ts(it, P), :], in_=ob)

            def emit_chunk_scores(t0, ntl, u_tiles):
                for tk in range(t0 + ntl):
                    emit_scores(t0, ntl, tk, u_tiles)

            def emit_chunk_avs(t0, ntl, u_tiles, pre=None):
                # `pre`: thunks emitting the NEXT chunk's score tiles, woven
                # between AV sub-chunks so ACT keeps streaming across the
                # chunk boundary
                pre = pre or []
                pi = 0
                for s in range(ntl):
                    emit_av(t0, s, u_tiles)
                    it = t0 + s
                    if it > 0:
                        emit_proj_tile(it - 1)
                    take = (len(pre) * (s + 1)) // ntl - pi
                    for _ in range(take):
                        pre[pi]()
                        pi += 1

            raw_tiles = {}
            emit_warmup(CFG["warm"])
            emit_xg(0)
            emit_early_dmas()
            emit_xg(1)
            emit_group_front(0)
            emit_group_tr(0)
            emit_xg(2)
            emit_xg(3)
            emit_group_front(1)
            emit_late_dmas()
            u0, u1, u2, u3 = [], [], [], []
            emit_chunk_scores(0, 4, u0)
            emit_group_tr(1)
            emit_group_front(2)
            pre1 = [
                (lambda tk=tk: emit_scores(4, 4, tk, u1)) for tk in range(8)
            ]
            emit_chunk_avs(0, 4, u0, pre=pre1)
            emit_group_tr(2)
            emit_group_front(3)
            pre2 = [
                (lambda tk=tk: emit_scores(8, 4, tk, u2)) for tk in range(12)
            ]
            emit_chunk_avs(4, 4, u1, pre=pre2)
            emit_group_tr(3)
            pre3 = [
                (lambda tk=tk: emit_scores(12, 4, tk, u3)) for tk in range(12)
            ]
            emit_chunk_avs(8, 4, u2, pre=pre3)
            # final chunk: weave AVs into the diagonal score stream so the
            # AV/proj tail overlaps the exp wall instead of following it
            emit_proj_tile(11)
            for s in range(GT):
                emit_scores(12, 4, 12 + s, u3)
                emit_av(12, s, u3)
                if s > 0:
                    emit_proj_tile(12 + s - 1)
            emit_proj_tile(NT - 1)

    nc.finalize()
    return nc


_NC_CACHE = {}


def _get_nc():
    if "nc" not in _NC_CACHE:
        _NC_CACHE["nc"] = _build_bass()
    return _NC_CACHE["nc"]


def _make_blk(q_gain_local):
    blk = np.zeros((P, BLK_W + 2 * NCONST), dtype=np.float16)
    blk[:, 0:128] = np.eye(P, dtype=np.float32)
    tri = (np.arange(P)[None, :] >= np.arange(P)[:, None]).astype(np.float32)
    for j in range(4):
        blk[:, 128 + 128 * j : 256 + 128 * j] = tri
    # per-lane rstd fold factor (f32, bitcast into the f16 tail):
    # q lanes: q_gain * SCALE * sqrt(HD) = q_gain;  k lanes: sqrt(HD)
    lane = np.empty((NQK,), np.float32)
    for j in range(GT):
        lane[j * (NH + 1) : j * (NH + 1) + NH] = np.asarray(
            q_gain_local, np.float32
        )
        lane[j * (NH + 1) + NH] = np.sqrt(HD)
    qgk = np.broadcast_to(lane[None, :], (P, NQK)).astype(np.float32)
    blk[:, BLK_W : BLK_W + 2 * NCONST] = np.ascontiguousarray(qgk).view(
        np.float16
    )
    return blk


def _rope_tables():
    inv = 1.0 / (ROPE_BASE ** (np.arange(0, HD, 2, dtype=np.float32) / HD))
    f = np.arange(T, dtype=np.float32)[:, None] * inv[None, :]

    def shuf(a):
        a5 = np.broadcast_to(a[:, None, :], (T, NH + 1, HD // 2))
        return a5.reshape(NT, P, NH + 1, HD // 2).transpose(1, 0, 2, 3)

    rope = np.stack([shuf(np.cos(f)), shuf(np.sin(f))], axis=1)
    return np.ascontiguousarray(rope).astype(np.float16)


def _make_in_maps(x, Wq, Wk, Wv, Wproj, q_gain):
    x = np.ascontiguousarray(np.asarray(x, np.float32))
    Wq = np.asarray(Wq, np.float32)
    Wk = np.asarray(Wk, np.float32)
    Wv = np.asarray(Wv, np.float32)
    Wproj = np.asarray(Wproj, np.float32)
    q_gain = np.asarray(q_gain, np.float32)
    rope = _rope_tables()
    xTs = [np.ascontiguousarray(x[b].T.astype(np.float16)) for b in range(B)]
    kvw = NH * HD  # 256 per-core q slice width
    in_maps = []
    for core in range(8):
        b, g = divmod(core, HKV)
        wq = Wq[g * kvw : (g + 1) * kvw]
        wk = Wk[g * HD : (g + 1) * HD]
        wv = Wv[g * HD : (g + 1) * HD]
        wqkvT = np.ascontiguousarray(
            np.concatenate([wq, wk, wv], 0).T.astype(np.float16)
        )
        wprojT = np.ascontiguousarray(
            Wproj[:, g * kvw : (g + 1) * kvw].T.astype(np.float16)
        )
        in_maps.append(
            {
                "xT": xTs[b],
                "wqkvT": wqkvT,
                "wprojT": wprojT,
                "ropeT": rope,
                "blk": _make_blk(q_gain[g * NH : (g + 1) * NH]),
            }
        )
    return in_maps


def run_sharded(inputs, trace=False, **kwargs):
    """Run the SPMD kernel; returns (full_output, BassKernelResults)."""
    in_maps = _make_in_maps(**inputs)
    res = run_bass_kernel_spmd(
        _get_nc(), in_maps, core_ids=list(range(8)), trace=trace, **kwargs
    )
    out = np.zeros((B, T, D), np.float32)
    for core in range(8):
        out[core // HKV] += res.results[core]["outp"].astype(np.float32)
    return out, res


def kernel(x, Wq, Wk, Wv, Wproj, q_gain):
    out, _ = run_sharded(
        dict(x=x, Wq=Wq, Wk=Wk, Wv=Wv, Wproj=Wproj, q_gain=q_gain)
    )
    return out


# revision 67
# speedup vs baseline: 1.2450x; 1.0626x over previous
"""Causal self-attention (GQA, qk-RMS-norm, RoPE) Trainium2 Bass kernel.

Sharding (8 cores): batch (2) x kv-head-group (4).  Core c handles batch
b = c // 4 and kv head g = c % 4 (with its 4 query heads 4g..4g+3).
Each core computes a (T, D) partial of the output projection (Wproj input
dim is split across the 4 tensor-parallel cores); the host sums the 4
fp16 partials per batch element in f32.

v4 design (everything fp16 device-side; f32 only in PSUM and stats):
  - QKV proj: psum[t_tile, 384] = sum_k xT_tile^T . wqkvT_tile; the
    wqkv/x input DMAs are split so the first accumulation starts early
  - rms stats on DVE from the fp16 copy of raw q/k (squares + reduce +
    bit-trick rsqrt + 2 Newton steps); rstd * q_gain * SCALE folded into
    the fp16 cast of q/k; eps folded as HD*eps on ssq
  - rope applied in place on raw q/k (DVE 2x; groups 2-3 on gpsimd)
  - k is cast into TWO adjacent lanes so a single [128,128] PE transpose
    lands kT duplicated across both partition halves; per-group
    transposes pack into one psum bank and drain with ONE batched copy
  - scores computed transposed S^T[tk, tq] (|s| <= 8 after rms norm, so
    no max pass); exp straight out of PSUM on ACT (a t~0 primer
    activation forces the exp-table load off the critical path);
    multiplicative {0,1} tri-mask on the diagonal 128-block (DVE)
  - A.V per (head, tq-tile, tk-tile) with rhs = [V | ones] (65 cols)
    accumulating y AND the softmax denominator in one psum bank;
    normalization is a per-partition reciprocal multiply on the
    psum->sbuf cast; y head-pairs are PE-transposed to yT for the
    output projection
  - schedule: chunk score/exp streams overlap the next chunk via
    score-prefetch thunks woven between AV sub-chunks; group fronts
    (qkv+stats) and transposes are placed so their DVE chains hide
    under the previous chunk's exp stream; the final chunk weaves its
    diagonal scores/AVs/projs so the tail after the last exp is minimal
  - engine balance (vs the TimelineSim cost model): ACT = exp stream
    (~95% saturated mid-kernel) + early qk_raw evacuations; DVE =
    stats/masks/normalize/psum evacuations; rope is lane-split across
    DVE+gpsimd; gpsimd also takes 1/3 of the rstd casts; PE ~87us busy,
    span ~122.5us (head ~21 DMA+chain gated, exp stream ~89, tail ~12)
"""
import os
import sys

import numpy as np

for _p in ("/opt/trn_rl_repo", "/root/.axon_site/_ro/trn_rl_repo"):
    if _p not in sys.path and os.path.isdir(_p):
        sys.path.append(_p)

import concourse.bass as bass
import concourse.bacc as bacc_mod
import concourse.mybir as mybir
import concourse.tile as tile
from concourse.bass import ts
from concourse.bass_utils import run_bass_kernel_spmd

F32 = mybir.dt.float32
F16 = mybir.dt.float16
I16 = mybir.dt.int16
I32 = mybir.dt.int32

B, T, D = 2, 2048, 1024
H, HKV, HD = 16, 4, 64
NH = H // HKV            # 4 local q heads per core
P = 128
NT = T // P              # 16 t tiles
ND = D // P              # 8 d tiles
GT = 4                   # t-tiles per qkv group
NG = NT // GT            # 4 groups
CW = 512                 # tq chunk width
NCHUNK = T // CW         # 4
QKV_W = NH * HD + 2 * HD  # 384 = q(256) | k(64) | v(64)
NQK = GT * (NH + 1)      # 20 rms/rope lanes per group
EPS = float(np.finfo(np.float32).eps)
SCALE = float(HD) ** -0.5
ROPE_BASE = 10000.0
RSQRT_MAGIC = 0x5F3759DF

NCONST = NQK
BLK_W = 640

# ---- engine-assignment knobs (tuned against the TimelineSim cost model) ----
CFG = {
    "warm": 6,
    "rope_pool_groups": (2, 3),      # groups whose rope runs on gpsimd
    "mask_pool_chunks": (),      # chunks whose tri-mask runs on gpsimd
    "ob_act_tiles": (),          # t-tiles whose first ob half copies on ACT
    "ob_act_both_tiles": (),  # second half on ACT (tail)
    "ynorm_act_tiles": (),       # t-tiles whose y-normalize runs on ACT
    # chunks (by t0) whose pr=1 exp runs as the DVE fp16 int-exp
    # (2^x bit trick; ~2% element rms err on those weights)
    "intexp_chunks": (),
    # psum pool shape: "s2y2m2" (baseline), "s3y1m1" (3rd score slot),
    # "qkvown" (dedicated qkv pool out of the score rotation, y1 m1)
    "psum_mode": "s2y2m2",
}

# int-exp constants: i16 = s * 1024*log2e + (15360 - 44 + 0.5) -> f16 bits
INTEXP_C = 1024.0 / float(np.log(2.0))
INTEXP_B = 15316.5


def _build_bass():
    nc = bacc_mod.Bacc(trn_type="TRN2")

    xT_d = nc.dram_tensor("xT", [D, T], F16, kind="ExternalInput")
    wqkv_d = nc.dram_tensor("wqkvT", [D, QKV_W], F16, kind="ExternalInput")
    wproj_d = nc.dram_tensor("wprojT", [NH * HD, D], F16, kind="ExternalInput")
    rope_d = nc.dram_tensor(
        "ropeT", [P, 2, NT, NH + 1, HD // 2], F16, kind="ExternalInput"
    )
    blk_d = nc.dram_tensor("blk", [P, BLK_W + 2 * NCONST], F16,
                           kind="ExternalInput")
    out_d = nc.dram_tensor("outp", [T, D], F16, kind="ExternalOutput")

    with tile.TileContext(nc) as tc:
        with (
            tc.tile_pool(name="singles", bufs=1) as singles,
            tc.tile_pool(name="xg", bufs=4) as xg_pool,
            tc.tile_pool(name="qk", bufs=2) as qk_pool,
            tc.tile_pool(name="stat", bufs=2) as stat_pool,
            tc.tile_pool(name="u", bufs=24) as u_pool,
            tc.tile_pool(name="r", bufs=2) as r_pool,
            tc.tile_pool(name="ysb", bufs=2) as ysb_pool,
            tc.tile_pool(name="ob", bufs=2) as ob_pool,
            tc.tile_pool(
                name="s_ps",
                bufs=3 if CFG["psum_mode"] == "s3y1m1" else 2,
                space="PSUM",
            ) as s_pool,
            tc.tile_pool(
                name="q_ps",
                bufs=1,
                space="PSUM",
            ) as qkv_psum_pool,
            tc.tile_pool(
                name="y_ps",
                bufs=2 if CFG["psum_mode"] == "s2y2m2" else 1,
                space="PSUM",
            ) as y_pool,
            tc.tile_pool(
                name="m_ps",
                bufs=2 if CFG["psum_mode"] == "s2y2m2" else 1,
                space="PSUM",
            ) as m_pool,
        ):
            # ---------------- persistent SBUF ----------------
            wqkv_sb = singles.tile([P, ND, QKV_W], F16)
            wproj_sb = singles.tile([P, 2, D], F16)
            rope_sb = singles.tile([P, 2, NT, NH + 1, HD // 2], F16)
            blk_sb = singles.tile([P, BLK_W + 2 * NCONST], F16)
            magic_sb = singles.tile([P, NQK], I32)
            # pair pr: head 2pr at partitions 0:64, head 2pr+1 at 64:128
            qT_sb = singles.tile([P, 2, T], F16)
            kT_sb = singles.tile([P, T], F16)      # dup'd into both halves
            v_sb = singles.tile([P, NT, 66], F16)  # cols 0:64 V, col 64 ones
            yT_sb = singles.tile([P, 2, T], F16)

            # split: first 4 d-tiles unblock the first qkv accumulations
            nc.sync.dma_start(
                out=wqkv_sb[:, 0:4, :],
                in_=wqkv_d[0 : 4 * P, :].rearrange(
                    "(po pi) f -> pi po f", pi=P
                ),
            )


            def emit_late_dmas():
                nc.sync.dma_start(
                    out=wproj_sb,
                    in_=wproj_d[:].rearrange("(po pi) f -> pi po f", pi=P),
                )

            qgk = blk_sb[:, BLK_W : BLK_W + 2 * NCONST].bitcast(F32)
            ident = blk_sb[:, 0:128]
            tri4 = blk_sb[:, 128:640].rearrange("p (j f) -> p j f", j=4)

            # PE p-state warmup: junk matmuls keep PE busy until the first
            # xg tile lands so all real matmuls price at the full clock.
            warm_sb = singles.tile([P, CW], F16)
            nc.gpsimd.memset(warm_sb, 0.0)
            nc.gpsimd.memset(magic_sb, RSQRT_MAGIC)
            nc.gpsimd.memset(v_sb, 0.0)
            nc.gpsimd.memset(v_sb[:, :, 64:65], 1.0)
            # primer: forces the ACT exp-table load at t~0 instead of right
            # before the first real exp
            prime_sb = singles.tile([P, 1], F16)
            nc.scalar.square(prime_sb, warm_sb[:, 0:1])

            def emit_warmup(n):
                wps = s_pool.tile([P, 2, CW], F32, tag="s", name="warm")
                for i in range(n):
                    nc.tensor.matmul(
                        wps[:, i % 2, :],
                        lhsT=warm_sb[:, 0:128],
                        rhs=warm_sb,
                        start=True,
                        stop=True,
                        skip_group_check=True,
                    )

            xg_tiles = {}

            def emit_xg(g, split=False):
                xg_sb = xg_pool.tile([P, ND, GT * P], F16, tag="xg", name=f"xg{g}")
                if split:
                    nc.sync.dma_start(
                        out=xg_sb[:, 0:4, 0 : 2 * P],
                        in_=xT_d[0 : 4 * P, g * GT * P : g * GT * P + 2 * P]
                        .rearrange("(po pi) t -> pi po t", pi=P),
                    )
                    nc.sync.dma_start(
                        out=xg_sb[:, 4:8, 0 : 2 * P],
                        in_=xT_d[4 * P : 8 * P,
                                 g * GT * P : g * GT * P + 2 * P]
                        .rearrange("(po pi) t -> pi po t", pi=P),
                    )
                    nc.sync.dma_start(
                        out=wqkv_sb[:, 4:8, :],
                        in_=wqkv_d[4 * P : 8 * P, :].rearrange(
                            "(po pi) f -> pi po f", pi=P
                        ),
                    )
                    nc.sync.dma_start(
                        out=xg_sb[:, :, 2 * P : 4 * P],
                        in_=xT_d[:, g * GT * P + 2 * P : (g + 1) * GT * P]
                        .rearrange("(po pi) t -> pi po t", pi=P),
                    )
                else:
                    nc.sync.dma_start(
                        out=xg_sb,
                        in_=xT_d[:, ts(g, GT * P)].rearrange(
                            "(po pi) t -> pi po t", pi=P
                        ),
                    )
                xg_tiles[g] = xg_sb

            qkc_tiles = {}

            def emit_group_front(g, halves=(0, 1), parts=("qkv", "stats")):
                """QKV projection + rms stats + rope + fp16 cast (with the
                rstd/q_gain/SCALE folds) for group g (optionally one half =
                2 t-tiles at a time for head-latency pipelining)."""
                xg_sb = xg_tiles[g]
                if 0 in halves:
                    qkc_tiles[g] = qk_pool.tile(
                        [P, GT, NH + 2, HD], F16, tag="qkc",
                        name=f"qkc{g}"
                    )
                    raw_tiles[g] = (
                        qk_pool.tile([P, NQK, HD], F16, tag="qkraw",
                                     name=f"qkraw{g}"),
                        qk_pool.tile([P, NQK, HD], F16, tag="sq",
                                     name=f"sq{g}"),
                        stat_pool.tile([P, NQK], F32, tag="ssq",
                                       name=f"ssq{g}"),
                        stat_pool.tile([P, NQK], F32, tag="rstd",
                                       name=f"rstd{g}"),
                        stat_pool.tile([P, NQK], F32, tag="nt",
                                       name=f"nt{g}"),
                    )
                qk_c = qkc_tiles[g]
                qk_raw, sq, ssq_f, rstd_f, nt_f = raw_tiles[g]
                for half in (halves if "qkv" in parts else ()):
                    qpool = (qkv_psum_pool if CFG["psum_mode"] == "qkvown"
                             else s_pool)
                    qkv_ps = qpool.tile([P, 2, 512], F32, tag="s",
                                        name=f"qkv{g}_{half}")
                    for jj in range(2):
                        j = 2 * half + jj
                        for ik in range(ND):
                            nc.tensor.matmul(
                                qkv_ps[:, jj, 0:QKV_W],
                                lhsT=xg_sb[:, ik, ts(j, P)],
                                rhs=wqkv_sb[:, ik, :],
                                start=(ik == 0),
                                stop=(ik == ND - 1),
                            )
                    h0 = 2 * half * (NH + 1)
                    hn = 2 * (NH + 1)
                    ceng = nc.scalar if g < 2 else nc.vector
                    if g < 2:
                        nc.scalar.copy(
                            qk_raw[:, h0 : h0 + hn, :].rearrange(
                                "p n x -> p (n x)"
                            ),
                            qkv_ps[:, :, 0 : (NH + 1) * HD],
                        )
                    else:
                        nc.vector.tensor_copy(
                            qk_raw[:, h0 : h0 + hn, :].rearrange(
                                "p n x -> p (n x)"
                            ),
                            qkv_ps[:, :, 0 : (NH + 1) * HD],
                        )
                    nc.vector.tensor_copy(
                        v_sb[:, ts(2 * g + half, 2), 0:64],
                        qkv_ps[:, :, 320:384],
                    )

                for half in (halves if "stats" in parts else ()):
                    h0 = 2 * half * (NH + 1)
                    hn = 2 * (NH + 1)
                    # squares on DVE from the fp16 copy (frees the qkv psum
                    # slot sooner and keeps the ACT queue exp-only)
                    nc.vector.tensor_mul(
                        sq[:, h0 : h0 + hn, :], qk_raw[:, h0 : h0 + hn, :],
                        qk_raw[:, h0 : h0 + hn, :],
                    )
                    # hierarchical: fp16 pair-add halves (2x mode), then a
                    # 32-wide reduce -- cheaper than one 64-wide reduce
                    nc.vector.tensor_add(
                        sq[:, h0 : h0 + hn, 0 : HD // 2],
                        sq[:, h0 : h0 + hn, 0 : HD // 2],
                        sq[:, h0 : h0 + hn, HD // 2 : HD],
                    )
                    ssq = ssq_f[:, h0 : h0 + hn]
                    nc.vector.reduce_sum(
                        ssq, sq[:, h0 : h0 + hn, 0 : HD // 2],
                        axis=mybir.AxisListType.X,
                    )
                    nc.vector.tensor_scalar_add(
                        out=ssq, in0=ssq, scalar1=float(HD * EPS)
                    )
                    # rstd/sqrt(HD) = rsqrt(ssq): bit-trick + 2 Newton steps
                    rstd = rstd_f[:, h0 : h0 + hn]
                    nt_ = nt_f[:, h0 : h0 + hn]
                    rstd_i = rstd.bitcast(I32)
                    nc.vector.tensor_scalar(
                        out=rstd_i,
                        in0=ssq.bitcast(I32),
                        scalar1=1,
                        scalar2=None,
                        op0=mybir.AluOpType.logical_shift_right,
                    )
                    nc.vector.tensor_sub(
                        rstd_i, magic_sb[:, h0 : h0 + hn], rstd_i
                    )
                    for _ in range(2):
                        nc.vector.tensor_mul(nt_, rstd, rstd)
                        nc.vector.tensor_mul(nt_, nt_, ssq)
                        nc.vector.tensor_scalar(
                            out=nt_,
                            in0=nt_,
                            scalar1=-0.5,
                            scalar2=1.5,
                            op0=mybir.AluOpType.mult,
                            op1=mybir.AluOpType.add,
                        )
                        nc.vector.tensor_mul(rstd, rstd, nt_)
                    # fold q_gain (q lanes) / sqrt(HD) restore (all lanes)
                    nc.vector.tensor_mul(
                        rstd, rstd, qgk[:, h0 : h0 + hn]
                    )

                    # rope in place on raw q|k
                    rr = qk_raw[:, h0 : h0 + hn, :]
                    q1 = rr[:, :, 0 : HD // 2]
                    q2 = rr[:, :, HD // 2 : HD]
                    tsl = ts(2 * g + half, 2)
                    cg = rope_sb[:, 0:1, tsl, :, :].rearrange(
                        "p o g h x -> p (o g h) x"
                    )
                    sg = rope_sb[:, 1:2, tsl, :, :].rearrange(
                        "p o g h x -> p (o g h) x"
                    )
                    t_a = qk_pool.tile([P, hn, HD // 2], F16, tag="ta",
                                       name=f"ta{g}_{half}")
                    t_b = qk_pool.tile([P, hn, HD // 2], F16, tag="tb",
                                       name=f"tb{g}_{half}")
                    t_c = qk_pool.tile([P, hn, HD // 2], F16, tag="tc",
                                       name=f"tc{g}_{half}")
                    t_d = qk_pool.tile([P, hn, HD // 2], F16, tag="td",
                                       name=f"td{g}_{half}")
                    if g in CFG["rope_pool_groups"]:
                        # lane-split rope: j-even lanes on DVE, j-odd on
                        # gpsimd -- two short parallel chains instead of one
                        # long serial chain on the slow engine
                        hh2 = hn // 2
                        for eng, sl in ((nc.vector, slice(0, hh2)),
                                        (nc.gpsimd, slice(hh2, hn))):
                            eng.tensor_mul(t_a[:, sl, :], q1[:, sl, :],
                                           cg[:, sl, :])
                            eng.tensor_mul(t_b[:, sl, :], q2[:, sl, :],
                                           sg[:, sl, :])
                            eng.tensor_mul(t_c[:, sl, :], q1[:, sl, :],
                                           sg[:, sl, :])
                            eng.tensor_mul(t_d[:, sl, :], q2[:, sl, :],
                                           cg[:, sl, :])
                            eng.tensor_add(q1[:, sl, :], t_a[:, sl, :],
                                           t_b[:, sl, :])
                            eng.tensor_sub(q2[:, sl, :], t_d[:, sl, :],
                                           t_c[:, sl, :])
                    else:
                        nc.vector.tensor_mul(t_a, q1, cg)
                        nc.vector.tensor_mul(t_b, q2, sg)
                        nc.vector.tensor_mul(t_c, q1, sg)
                        nc.vector.tensor_mul(t_d, q2, cg)
                        nc.vector.tensor_add(q1, t_a, t_b)
                        nc.vector.tensor_sub(q2, t_d, t_c)

                    # scale into qk_c; k goes to TWO adjacent lanes so one
                    # [128,128] PE transpose lands kT dup'd in both halves
                    ndve = 0
                    for jj in range(2):
                        j = 2 * half + jj
                        i0 = j * (NH + 1)
                        for h in range(NH + 2):
                            src_lane = i0 + min(h, NH)
                            eng = nc.vector if (ndve % 2 == 0) else nc.gpsimd
                            ndve += 1
                            eng.tensor_scalar_mul(
                                out=qk_c[:, j, h, :],
                                in0=qk_raw[:, src_lane, :],
                                scalar1=rstd_f[:, src_lane : src_lane + 1],
                            )

            def emit_group_tr(g, halves=(0, 1)):
                """fp16 transposes: q-pair + dup'd-k transposes packed into
                psum tiles, then ONE batched DVE copy each."""
                qk_c = qkc_tiles[g]
                js = [2 * h + jj for h in halves for jj in range(2)]
                nj = len(js)
                trq = m_pool.tile([P, 2, nj * P], F16, tag="m",
                                  name=f"trq{g}_{halves[0]}")
                trk = m_pool.tile([P, nj * P], F16, tag="m",
                                  name=f"trk{g}_{halves[0]}")
                for i, j in enumerate(js):
                    for pr in range(2):
                        nc.tensor.transpose(
                            trq[:, pr, ts(i, P)],
                            qk_c[:, j, 2 * pr : 2 * pr + 2, :],
                            ident,
                        )
                    nc.tensor.transpose(
                        trk[:, ts(i, P)], qk_c[:, j, NH : NH + 2, :], ident
                    )
                t0 = g * GT + js[0]
                nc.vector.tensor_copy(
                    qT_sb[:, :, t0 * P : (t0 + nj) * P], trq
                )
                nc.vector.tensor_copy(
                    kT_sb[:, t0 * P : (t0 + nj) * P], trk
                )

            def emit_scores(t0, ntl, tk, u_tiles):
                """Score tile (tk) for the tq range [t0, t0+ntl) t-tiles."""
                dj = tk - t0  # >= 0 on the diagonal tiles
                lo = P * dj if dj >= 0 else 0
                w = ntl * P
                u = u_pool.tile([P, NH, CW], F16, tag="u",
                                name=f"u_t{t0}_{tk}")
                for pr in range(2):
                    s_ps = s_pool.tile([P, 2, CW], F32, tag="s")
                    for hh in range(2):
                        nc.tensor.matmul(
                            s_ps[:, hh, lo:w],
                            lhsT=kT_sb[64 * hh : 64 * (hh + 1), ts(tk, P)],
                            rhs=qT_sb[
                                64 * hh : 64 * (hh + 1),
                                pr,
                                t0 * P + lo : (t0 + ntl) * P,
                            ],
                            start=True,
                            stop=True,
                        )
                    if pr == 1 and t0 in CFG["intexp_chunks"]:
                        # exp(s) ~= bitcast_f16(i16(s*1024*log2e + B)):
                        # one DVE op; splits the exp cadence across ACT+DVE
                        nc.vector.tensor_scalar(
                            out=u[:, 2 * pr : 2 * pr + 2, lo:w].bitcast(I16),
                            in0=s_ps[:, :, lo:w],
                            scalar1=INTEXP_C,
                            scalar2=INTEXP_B,
                            op0=mybir.AluOpType.mult,
                            op1=mybir.AluOpType.add,
                        )
                    else:
                        nc.scalar.activation(
                            out=u[:, 2 * pr : 2 * pr + 2, lo:w],
                            in_=s_ps[:, :, lo:w],
                            func=mybir.ActivationFunctionType.Exp,
                        )
                if dj >= 0:
                    # multiplicative causal mask on the diagonal 128-block
                    meng = (nc.gpsimd if t0 in CFG["mask_pool_chunks"]
                            else nc.vector)
                    meng.tensor_mul(
                        u[:, :, lo : lo + P], u[:, :, lo : lo + P], tri4
                    )
                u_tiles.append(u)

            def emit_av(t0, s, u_tiles):
                """A.V + normalize + yT transpose for t-tile it = t0+s.
                Tail tiles run pair-granular: heads 0-1 are normalized,
                transposed and copied while heads 2-3 still accumulate, so
                the final projection starts one pair earlier."""
                it = t0 + s
                nk = it + 1
                pairs = 2 if it in CFG["pair_av_tiles"] else 1
                y_ps = y_pool.tile([P, NH, 128], F32, tag="y",
                                   name=f"y_t{it}")
                dr = r_pool.tile([P, NH], F32, tag="dr")
                y_sb = ysb_pool.tile([P, NH, HD], F16, tag="ysb")
                try_ = m_pool.tile([P, 2, 128], F16, tag="m",
                                   name=f"try_{it}")
                for pp in range(pairs):
                    hs = range(pp * NH // pairs, (pp + 1) * NH // pairs)
                    for h in hs:
                        for tk in range(nk):
                            nc.tensor.matmul(
                                y_ps[:, h, 0:65],
                                lhsT=u_tiles[tk][:, h, ts(s, P)],
                                rhs=v_sb[:, tk, 0:65],
                                start=(tk == 0),
                                stop=(tk == nk - 1),
                                skip_group_check=True,
                            )
                    h0, h1 = hs.start, hs.stop
                    nc.vector.reciprocal(
                        dr[:, h0:h1], y_ps[:, h0:h1, 64:65]
                    )
                    for h in hs:
                        if it in CFG["ynorm_act_tiles"]:
                            nc.scalar.activation(
                                out=y_sb[:, h, :],
                                in_=y_ps[:, h, 0:64],
                                func=mybir.ActivationFunctionType.Copy,
                                scale=dr[:, h : h + 1],
                            )
                        else:
                            nc.vector.tensor_scalar_mul(
                                out=y_sb[:, h, :],
                                in0=y_ps[:, h, 0:64],
                                scalar1=dr[:, h : h + 1],
                            )
                    for pr in range(h0 // 2, (h1 + 1) // 2):
                        nc.tensor.transpose(
                            try_[:, pr, :],
                            y_sb[:, 2 * pr : 2 * pr + 2, :],
                            ident,
                        )
                        if pairs == 2:
                            nc.vector.tensor_copy(
                                yT_sb[:, pr, ts(it, P)], try_[:, pr, :]
                            )
                if pairs == 1:
                    nc.vector.tensor_copy(yT_sb[:, :, ts(it, P)], try_)

            def emit_proj_tile(it):
                """Output projection for t-tile it -> fp16 partial.  The
                last tiles evacuate each half on a different engine and
                store per-half so the tail doesn't serialize on DVE."""
                tail = it in CFG["tail_split_tiles"]
                ob = ob_pool.tile([P, D], F16, tag="ob")
                for nh_ in range(2):
                    pj = m_pool.tile([P, 512], F32, tag="m",
                                     name=f"pj{it}_{nh_}")
                    for kt in range(2):
                        nc.tensor.matmul(
                            pj,
                            lhsT=yT_sb[:, kt, ts(it, P)],
                            rhs=wproj_sb[:, kt, ts(nh_, 512)],
                            start=(kt == 0),
                            stop=(kt == 1),
                        )
                    if tail and nh_ == 0:
                        nc.scalar.copy(ob[:, ts(nh_, 512)], pj)
                    elif it in CFG["ob_act_tiles"] and nh_ == 0:
                        nc.scalar.copy(ob[:, ts(nh_, 512)], pj)
                    else:
                        nc.vector.tensor_copy(ob[:, ts(nh_, 512)], pj)
                    if tail:
                        nc.sync.dma_start(
                            out=out_d[ts(it, P), ts(nh_, 512)],
                            in_=ob[:, ts(nh_, 512)],
                        )
                if not tail:
                    nc.sync.dma_start(out=out_d[ts(it, P), :], in_=ob)

            def emit_chunk_scores(t0, ntl, u_tiles):
                for tk in range(t0 + ntl):
                    emit_scores(t0, ntl, tk, u_tiles)

            def emit_chunk_avs(t0, ntl, u_tiles, pre=None):
                # `pre`: thunks emitting the NEXT chunk's score tiles, woven
                # between AV sub-chunks so ACT keeps streaming across the
                # chunk boundary
                pre = pre or []
                pi = 0
                ffrac = CFG.get("pre_front", 1)
                for s in range(ntl):
                    emit_av(t0, s, u_tiles)
                    it = t0 + s
                    if it > 0:
                        emit_proj_tile(it - 1)
                    take = min(len(pre),
                               (len(pre) * (s + 1) * ffrac) // ntl) - pi
                    for _ in range(take):
                        pre[pi]()
                        pi += 1

            raw_tiles = {}
            emit_warmup(CFG["warm"])
            emit_xg(0, split=True)
            nc.sync.dma_start(
                out=rope_sb[:, :, 0:GT, :, :],
                in_=rope_d[:, :, 0:GT, :, :],
            )
            emit_xg(1)
            nc.sync.dma_start(out=blk_sb, in_=blk_d[:])
            nc.sync.dma_start(
                out=rope_sb[:, :, GT:NT, :, :],
                in_=rope_d[:, :, GT:NT, :, :],
            )
            emit_group_front(0)
            emit_group_tr(0)
            emit_xg(2)
            emit_xg(3)
            emit_group_front(1)
            emit_late_dmas()
            u0, u1, u2, u3 = [], [], [], []
            emit_chunk_scores(0, 4, u0)
            emit_group_tr(1)
            emit_group_front(2)
            pre1 = [
                (lambda tk=tk: emit_scores(4, 4, tk, u1)) for tk in range(8)
            ]
            emit_chunk_avs(0, 4, u0, pre=pre1)
            emit_group_front(3)
            emit_group_tr(2)
            pre2 = [
                (lambda tk=tk: emit_scores(8, 4, tk, u2)) for tk in range(12)
            ]
            emit_chunk_avs(4, 4, u1, pre=pre2)
            emit_group_tr(3)
            pre3 = [
                (lambda tk=tk: emit_scores(12, 4, tk, u3)) for tk in range(12)
            ]
            emit_chunk_avs(8, 4, u2, pre=pre3)
            # final chunk: diagonal scores ASAP (exp-paced), AVs and projs
            # chase so the tail after the last exp is minimal
            emit_proj_tile(11)
            emit_scores(12, 4, 12, u3)
            emit_scores(12, 4, 13, u3)
            emit_av(12, 0, u3)
            emit_scores(12, 4, 14, u3)
            emit_av(12, 1, u3)
            emit_proj_tile(12)
            emit_scores(12, 4, 15, u3)
            emit_av(12, 2, u3)
            emit_proj_tile(13)
            emit_av(12, 3, u3)
            emit_proj_tile(14)
            emit_proj_tile(15)

    nc.finalize()
    return nc


_NC_CACHE = {}


def _get_nc():
    if "nc" not in _NC_CACHE:
        _NC_CACHE["nc"] = _build_bass()
    return _NC_CACHE["nc"]


def _make_blk(q_gain_local):
    blk = np.zeros((P, BLK_W + 2 * NCONST), dtype=np.float16)
    blk[:, 0:128] = np.eye(P, dtype=np.float32)
    tri = (np.arange(P)[None, :] >= np.arange(P)[:, None]).astype(np.float32)
    for j in range(4):
        blk[:, 128 + 128 * j : 256 + 128 * j] = tri
    # per-lane rstd fold factor (f32, bitcast into the f16 tail):
    # q lanes: q_gain * SCALE * sqrt(HD) = q_gain;  k lanes: sqrt(HD)
    lane = np.empty((NQK,), np.float32)
    for j in range(GT):
        lane[j * (NH + 1) : j * (NH + 1) + NH] = np.asarray(
            q_gain_local, np.float32
        )
        lane[j * (NH + 1) + NH] = np.sqrt(HD)
    qgk = np.broadcast_to(lane[None, :], (P, NQK)).astype(np.float32)
    blk[:, BLK_W : BLK_W + 2 * NCONST] = np.ascontiguousarray(qgk).view(
        np.float16
    )
    return blk


def _rope_tables():
    inv = 1.0 / (ROPE_BASE ** (np.arange(0, HD, 2, dtype=np.float32) / HD))
    f = np.arange(T, dtype=np.float32)[:, None] * inv[None, :]

    def shuf(a):
        a5 = np.broadcast_to(a[:, None, :], (T, NH + 1, HD // 2))
        return a5.reshape(NT, P, NH + 1, HD // 2).transpose(1, 0, 2, 3)

    rope = np.stack([shuf(np.cos(f)), shuf(np.sin(f))], axis=1)
    return np.ascontiguousarray(rope).astype(np.float16)


def _make_in_maps(x, Wq, Wk, Wv, Wproj, q_gain):
    x = np.ascontiguousarray(np.asarray(x, np.float32))
    Wq = np.asarray(Wq, np.float32)
    Wk = np.asarray(Wk, np.float32)
    Wv = np.asarray(Wv, np.float32)
    Wproj = np.asarray(Wproj, np.float32)
    q_gain = np.asarray(q_gain, np.float32)
    rope = _rope_tables()
    xTs = [np.ascontiguousarray(x[b].T.astype(np.float16)) for b in range(B)]
    kvw = NH * HD  # 256 per-core q slice width
    in_maps = []
    for core in range(8):
        b, g = divmod(core, HKV)
        wq = Wq[g * kvw : (g + 1) * kvw]
        wk = Wk[g * HD : (g + 1) * HD]
        wv = Wv[g * HD : (g + 1) * HD]
        wqkvT = np.ascontiguousarray(
            np.concatenate([wq, wk, wv], 0).T.astype(np.float16)
        )
        wprojT = np.ascontiguousarray(
            Wproj[:, g * kvw : (g + 1) * kvw].T.astype(np.float16)
        )
        in_maps.append(
            {
                "xT": xTs[b],
                "wqkvT": wqkvT,
                "wprojT": wprojT,
                "ropeT": rope,
                "blk": _make_blk(q_gain[g * NH : (g + 1) * NH]),
            }
        )
    return in_maps


def run_sharded(inputs, trace=False, **kwargs):
    """Run the SPMD kernel; returns (full_output, BassKernelResults)."""
    in_maps = _make_in_maps(**inputs)
    res = run_bass_kernel_spmd(
        _get_nc(), in_maps, core_ids=list(range(8)), trace=trace, **kwargs
    )
    out = np.zeros((B, T, D), np.float32)
    for core in range(8):
        out[core // HKV] += res.results[core]["outp"].astype(np.float32)
    return out, res


def kernel(x, Wq, Wk, Wv, Wproj, q_gain):
    out, _ = run_sharded(
        dict(x=x, Wq=Wq, Wk=Wk, Wv=Wv, Wproj=Wproj, q_gain=q_gain)
    )
    return out
